# revision 1
# baseline (speedup 1.0000x reference)
"""CorefHead Trainium2 kernel.

Reference computation (B=64, S=512, H=1024, HID=512):
  emb_a = span_mean(bert, offsets[:,0:2])   # [B,H]
  emb_b = span_mean(bert, offsets[:,2:4])   # [B,H]
  emb_p = bert[b, offsets[:,4]]             # [B,H]
  x = concat([emb_a, emb_b, emb_p], -1)     # [B,3H]
  h = leaky_relu(batchnorm_eval(x @ W1 + b1), 0.01)
  out = h @ W2 + b2                         # [B,3]

Strategy: pure data parallel, batch sharded 8 ways (8 batches/core).
Per core:
  - Host precomputes prescaled span masks (1/len) + pron one-hot, packed
    [128, nch, 3]; host also packs only the needed row-window of bert per
    batch (union of span/pron rows, padded to 128-row chunks).
  - mm1 (PE): xT[h_chunk] += bert_chunk.T @ mask_chunk -> x transposed
    [3072, 8] directly (no on-device transpose needed).
  - mm2 (PE): h[8, 512] += xT_chunk.T @ W1_chunk over 24 K-chunks.
  - BN+LeakyReLU (DVE): y = max(t, 0.01*t), t = h*scale + bias with
    scale/bias folded from (b1, gamma, beta, running stats) on host.
  - mm3 (DVE): out[:, j] = b2[j] + sum(y * W2[:, j]) via tensor_tensor_reduce.
Host gathers per-core [8, 3] outputs and undoes the batch permutation.
"""

import numpy as np

B, S, H = 64, 512, 1024
HID = 512
EPS = 1e-5
NCORES = 8
BPC = B // NCORES  # batches per core
KC = 3 * H // 128  # 24 contraction chunks for mm2
HC = H // 128      # 8 h-chunks per embedding

# Set to True to ship only the needed row-window of bert per batch.
WINDOW = True
# bert/masks in bf16: halves the dominant DMA traffic and runs mm1 single
# pass with fast weight load on the PE. Masks hold exact 0/1 (bf16-exact);
# the 1/span_len scaling happens in fp32 on the PSUM->SBUF copy.
BERT_BF16 = True
# W1 (and the mm2 x operand) in bf16.
W1_BF16 = True
# Ship pron rows fp32 + transpose on device. Only buys precision when the
# mm2 operands stay fp32 (otherwise x is rounded to bf16 anyway).
PRON_FP32 = not W1_BF16

# Test-harness hooks (harness calls kernel() with TRACE=False default).
TRACE = False
LAST_RESULT = None

_PROGRAM_CACHE: dict = {}


def _build_program(nch_slots: tuple):
    """Build + compile the SPMD Bass program for the given per-slot chunk
    counts (number of 128-row S-chunks shipped per batch slot)."""
    import concourse.bacc as bacc
    import concourse.tile as tile
    import concourse.mybir as mybir
    from concourse.bass import MemorySpace

    f32 = mybir.dt.float32
    bdt = mybir.dt.bfloat16 if BERT_BF16 else f32
    wdt = mybir.dt.bfloat16 if W1_BF16 else f32
    ncht = int(sum(nch_slots))
    totrows = ncht * 128

    nc = bacc.Bacc("TRN2", target_bir_lowering=False, debug=False,
                   num_devices=NCORES)

    nmc = 2 if PRON_FP32 else 3  # mask columns (spans only, or spans+pron)

    bert_d = nc.dram_tensor("bertw", [totrows, H], bdt, kind="ExternalInput").ap()
    mask_d = nc.dram_tensor("maskp", [128, ncht, nmc], bdt, kind="ExternalInput").ap()
    sfac_d = nc.dram_tensor("sfac", [128, BPC, nmc], f32, kind="ExternalInput").ap()
    if PRON_FP32:
        pron_d = nc.dram_tensor("pron", [BPC, H], f32, kind="ExternalInput").ap()
    w1_d = nc.dram_tensor("w1", [3 * H, HID], wdt, kind="ExternalInput").ap()
    # bnbP[p, mc] = folded BN bias for hid index mc*128+p; w2P[p, mc, j] =
    # W2[mc*128+p, j]; b2c = b2[:, None]
    bnb_d = nc.dram_tensor("bnbP", [128, HID // 128], f32,
                           kind="ExternalInput").ap()
    w2_d = nc.dram_tensor("w2P", [128, HID // 128, 3], f32,
                          kind="ExternalInput").ap()
    b2_d = nc.dram_tensor("b2c", [3, 1], f32, kind="ExternalInput").ap()
    out_d = nc.dram_tensor("out", [3, BPC], f32, kind="ExternalOutput").ap()

    chbase = np.concatenate([[0], np.cumsum(nch_slots)]).astype(int)

    with tile.TileContext(nc) as tc:
        with (
            tc.tile_pool(name="singles", bufs=1) as singles,
            tc.tile_pool(name="bert_pool", bufs=3) as bert_pool,
            tc.tile_pool(name="head", bufs=1) as head,
            tc.tile_pool(name="psum_x", bufs=4, space=MemorySpace.PSUM) as psum_x_pool,
            tc.tile_pool(name="psum_p", bufs=2, space=MemorySpace.PSUM) as psum_p_pool,
            tc.tile_pool(name="psum_h", bufs=1, space=MemorySpace.PSUM) as psum_h_pool,
        ):
            # --- constant / parameter loads ---
            mask_t = singles.tile([128, ncht, nmc], bdt)
            nc.scalar.dma_start(out=mask_t, in_=mask_d)
            sfac_t = singles.tile([128, BPC, nmc], f32)
            nc.scalar.dma_start(out=sfac_t, in_=sfac_d)
            if PRON_FP32:
                pron_t = singles.tile([BPC, H], f32)
                nc.scalar.dma_start(out=pron_t, in_=pron_d)
            bnb_t = head.tile([128, HID // 128], f32)
            nc.scalar.dma_start(out=bnb_t, in_=bnb_d)
            w2_t = head.tile([128, HID // 128, 3], f32)
            nc.scalar.dma_start(out=w2_t, in_=w2_d)
            b2_t = head.tile([3, 1], f32)
            nc.scalar.dma_start(out=b2_t, in_=b2_d)
            from concourse.masks import make_identity
            idt = singles.tile([BPC, BPC], f32)
            make_identity(nc, idt)

            # xT accumulator: [128, 3 embeddings, HC chunks * BPC batches]
            # column for contraction-chunk kc=(e*HC+hc), batch b is
            # xT[:, e, hc*BPC + b]  -> mm2 rhs slice [128, BPC] contiguous.
            xT_t = singles.tile([128, 3, HC * BPC], wdt)

            if PRON_FP32:
                # --- pron embedding: exact fp32 rows, transposed via PE ---
                for hc in range(HC):
                    pxp = psum_p_pool.tile([128, BPC], f32, tag="pxp")
                    nc.tensor.transpose(
                        pxp, pron_t[:, hc * 128:(hc + 1) * 128], idt)
                    nc.vector.tensor_copy(
                        xT_t[:, 2, hc * BPC:(hc + 1) * BPC], pxp)

            # --- mm1: per-batch span sums (bert chunks as PE weights) ---
            # W1 + consts ride the ACT HWDGE ring; berts ride the SP ring.
            # The SDMA engines round-robin between the rings, so the bert
            # stream is never stuck behind the big W1 transfer.
            w1_t = singles.tile([128, KC, HID], wdt)
            w1_src = w1_d.rearrange("(kc p) n -> p kc n", p=128)
            for i in range(4):
                nc.scalar.dma_start(
                    out=w1_t[:, 6 * i:6 * (i + 1), :],
                    in_=w1_src[:, 6 * i:6 * (i + 1), :])
            # bert slots DMA'd in pairs (~1.3-2.1 MB per transfer) for
            # better SDMA efficiency while still overlapping mm1.
            for pair in range(BPC // 2):
                b0 = 2 * pair
                nchp = int(nch_slots[b0]) + int(nch_slots[b0 + 1])
                bt = bert_pool.tile([128, nchp, H], bdt, tag="bert")
                r0 = int(chbase[b0]) * 128
                nc.sync.dma_start(
                    out=bt[:, :nchp, :],
                    in_=bert_d[r0:r0 + nchp * 128, :].rearrange(
                        "(sc p) h -> p sc h", p=128))
                for b in (b0, b0 + 1):
                    nch = int(nch_slots[b])
                    sc0 = int(chbase[b]) - int(chbase[b0])
                    for hc in range(HC):
                        px = psum_x_pool.tile([128, nmc], f32)
                        for sc in range(nch):
                            nc.tensor.matmul(
                                px,
                                bt[:, sc0 + sc, hc * 128:(hc + 1) * 128],
                                mask_t[:, int(chbase[b]) + sc, :],
                                start=(sc == 0),
                                stop=(sc == nch - 1),
                            )
                        # fp32 scale by (1/lenA, 1/lenB[, 1]), PSUM->SBUF
                        nc.vector.tensor_mul(
                            xT_t[:, 0:nmc, hc * BPC + b], px, sfac_t[:, b, :])

            # --- mm2: h[BPC, HID] = x @ (W1 * bn_scale) over 24 K-chunks ---
            # (the BN eval-mode scale is folded into W1 on the host)
            ph = psum_h_pool.tile([BPC, HID], f32)
            for kc in range(KC):
                e, hc = kc // HC, kc % HC
                nc.tensor.matmul(
                    ph,
                    xT_t[:, e, hc * BPC:(hc + 1) * BPC],
                    w1_t[:, kc, :],
                    start=(kc == 0),
                    stop=(kc == KC - 1),
                )
            hs_t = head.tile([BPC, HID], f32)
            nc.vector.tensor_copy(hs_t, ph)

            # --- per hid-chunk: transpose h, + BN bias, LeakyReLU, mm3 ---
            ot_ps = psum_h_pool.tile([3, BPC], f32, tag="oT")
            for mc in range(HID // 128):
                pht = psum_p_pool.tile([128, BPC], f32, tag="pht")
                nc.tensor.transpose(
                    pht, hs_t[:, mc * 128:(mc + 1) * 128], idt)
                t_t = head.tile([128, BPC], f32, tag="t_t")
                nc.vector.tensor_scalar_add(t_t, pht, bnb_t[:, mc:mc + 1])
                y_t = head.tile([128, BPC], f32, tag="y_t")
                # y = max(0.01 * t, t)
                nc.vector.scalar_tensor_tensor(
                    y_t, t_t, 0.01, t_t,
                    op0=mybir.AluOpType.mult, op1=mybir.AluOpType.max)
                nc.tensor.matmul(
                    ot_ps, w2_t[:, mc, :], y_t,
                    start=(mc == 0), stop=(mc == HID // 128 - 1))

            o_t = head.tile([3, BPC], f32)
            nc.vector.tensor_scalar_add(o_t, ot_ps, b2_t)
            nc.sync.dma_start(out=out_d, in_=o_t)

    nc.compile()
    return nc


def _prep_core_inputs(bert, bert_f32, offsets, w1, bnbP, w2P, b2c,
                      batch_idx, nch_slots):
    """Build the per-core input map for the given batch indices."""
    nmc = 2 if PRON_FP32 else 3
    ncht = int(sum(nch_slots))
    bertw = np.empty((ncht * 128, H), dtype=bert.dtype)
    maskp = np.zeros((128, ncht, nmc), dtype=bert.dtype)
    sfac = np.ones((BPC, nmc), dtype=np.float32)
    pron = np.empty((BPC, H), dtype=np.float32)
    row = 0
    for slot, gb in enumerate(batch_idx):
        nch = int(nch_slots[slot])
        L = nch * 128
        a0, a1, b0, b1_, p = (int(v) for v in offsets[gb])
        lo = min(a0, b0, p)
        w0 = max(0, min(lo, S - L))
        bertw[row:row + L] = bert[gb, w0:w0 + L]
        pron[slot] = bert_f32[gb, p]
        pos = w0 + np.arange(L)
        cols = [((pos >= a0) & (pos <= a1)).astype(np.float32),
                ((pos >= b0) & (pos <= b1_)).astype(np.float32)]
        if not PRON_FP32:
            cols.append((pos == p).astype(np.float32))
        sfac[slot, 0] = 1.0 / (a1 - a0 + 1)
        sfac[slot, 1] = 1.0 / (b1_ - b0 + 1)
        blk = np.stack(cols, axis=-1).reshape(nch, 128, nmc)
        maskp[:, row // 128:row // 128 + nch, :] = blk.transpose(1, 0, 2)
        row += L
    in_map = {
        "bertw": bertw,
        "maskp": maskp,
        "sfac": np.broadcast_to(sfac, (128, BPC, nmc)).copy(),
        "w1": w1,
        "bnbP": bnbP,
        "w2P": w2P,
        "b2c": b2c,
    }
    if PRON_FP32:
        in_map["pron"] = pron
    return in_map


def kernel(bert_outputs, offsets, W1, b1, gamma, beta, running_mean,
           running_var, W2, b2):
    import ml_dtypes

    bert_f32 = np.ascontiguousarray(np.asarray(bert_outputs, dtype=np.float32))
    bert = bert_f32.astype(ml_dtypes.bfloat16) if BERT_BF16 else bert_f32
    offs = np.asarray(offsets).astype(np.int64)
    W1 = np.asarray(W1, dtype=np.float32)
    b1 = np.asarray(b1, dtype=np.float32)
    gamma = np.asarray(gamma, dtype=np.float32)
    beta = np.asarray(beta, dtype=np.float32)
    rm = np.asarray(running_mean, dtype=np.float32)
    rv = np.asarray(running_var, dtype=np.float32)
    W2 = np.asarray(W2, dtype=np.float32)
    b2 = np.asarray(b2, dtype=np.float32)

    # Fold BN eval-mode stats: bn(xW1 + b1) = x(W1*s) + ((b1 - mean)*s + beta)
    s = gamma / np.sqrt(rv + EPS)
    bias = (b1 - rm) * s + beta
    W1 = np.ascontiguousarray(W1 * s[None, :])
    if W1_BF16:
        W1 = W1.astype(ml_dtypes.bfloat16)
    bnbP = np.ascontiguousarray(bias.reshape(HID // 128, 128).T)
    w2P = np.ascontiguousarray(
        W2.reshape(HID // 128, 128, 3).transpose(1, 0, 2))
    b2c = np.ascontiguousarray(b2.reshape(3, 1))

    # Row windows: union of span/pron rows per batch, padded to 128-row
    # chunks. Sort batches by window size so same-slot batches across cores
    # share one (max) chunk count; undone at gather time.
    if WINDOW:
        lo = offs[:, [0, 2, 4]].min(axis=1)
        hi = offs[:, [1, 3, 4]].max(axis=1)
        lens = np.minimum((hi - lo + 128) // 128 * 128, S)
        order = np.argsort(-lens, kind="stable")
    else:
        lens = np.full(B, S, dtype=np.int64)
        order = np.arange(B)

    # slot i of every core holds batches ranked [i*NCORES, (i+1)*NCORES)
    perm = order.reshape(BPC, NCORES)  # [slot, core] -> global batch
    nch_slots = tuple(int(lens[perm[i]].max()) // 128 for i in range(BPC))

    key = nch_slots
    if key not in _PROGRAM_CACHE:
        _PROGRAM_CACHE[key] = _build_program(key)
    nc = _PROGRAM_CACHE[key]

    in_maps = [
        _prep_core_inputs(bert, bert_f32, offs, W1, bnbP, w2P, b2c,
                          perm[:, c], nch_slots)
        for c in range(NCORES)
    ]

    from concourse import bass_utils
    kwargs = {}
    if TRACE:
        kwargs = {"trace": True, "trace_cores": list(range(NCORES))}
    res = bass_utils.run_bass_kernel_spmd(nc, in_maps,
                                          core_ids=list(range(NCORES)),
                                          **kwargs)
    global LAST_RESULT
    LAST_RESULT = res

    out = np.empty((B, 3), dtype=np.float32)
    for c in range(NCORES):
        out[perm[:, c]] = res.results[c]["out"].T
    return out



# revision 5
# speedup vs baseline: 1.3339x; 1.3339x over previous
"""CorefHead Trainium2 kernel.

Reference computation (B=64, S=512, H=1024, HID=512):
  emb_a = span_mean(bert, offsets[:,0:2])   # [B,H]
  emb_b = span_mean(bert, offsets[:,2:4])   # [B,H]
  emb_p = bert[b, offsets[:,4]]             # [B,H]
  x = concat([emb_a, emb_b, emb_p], -1)     # [B,3H]
  h = leaky_relu(batchnorm_eval(x @ W1 + b1), 0.01)
  out = h @ W2 + b2                         # [B,3]

Strategy: pure data parallel, batch sharded 8 ways (8 batches/core),
DMA-byte and PE-cycle minimized:
  - Host ships ONLY the union of span rows per batch, packed back-to-back
    across the core's 8 batches (no per-batch chunk alignment); the pron
    rows are host-gathered/transposed into a tiny [128, 64] tile, so they
    never force extra 128-row chunks.
  - mm1 is flipped vs the obvious orientation: the 0/1 span masks
    [128, 16] are the stationary PE operand (LDWEIGHTS of 16 columns is
    ~free) and bert streams as the moving operand at N=512, accumulating
    ALL chunks into one PSUM pair [16, 1024] (cols = 2 spans x 8 slots).
    This cuts PE time ~4x vs loading each bert chunk as weights.
  - x is scaled by 1/span_len in fp32 on the PSUM->SBUF copy, transposed
    to contraction-major via 8 PE transposes, cast to bf16.
  - DMA order on the sync ring: bert pieces FIRST, folded W1 (bf16) LAST,
    so mm1 is never starved and mm2 chases the W1 stream; the only
    serial tail is the last W1 piece's matmuls + the tiny head.
  - mm2: ph[8, 512] += xT[:, kc, :].T @ W1x[:, kc, :] over 24 k-chunks.
  - Head: per 128-wide hid chunk: PE transpose, +BN bias (folded), leaky
    ReLU, mm3 accumulate [3, 8]; +b2; DMA out.
Host gathers per-core [3, 8] outputs and undoes the batch assignment.
"""

import numpy as np

B, S, H = 64, 512, 1024
HID = 512
EPS = 1e-5
NCORES = 8
BPC = B // NCORES   # batches per core
KC = 3 * H // 128   # 24 contraction chunks for mm2
HC = H // 128       # 8 h-chunks per embedding
NMC = 2 * BPC       # mm1 psum partitions: 2 spans x 8 slots

BERT_PIECE = 3      # bert chunks per DMA piece
W1_PIECE = 4        # w1 k-chunks per DMA piece

# Test-harness hooks (harness calls kernel() with TRACE=False default).
TRACE = False
LAST_RESULT = None

_PROGRAM_CACHE: dict = {}


def _build_program(nch: int):
    """Build + compile the SPMD Bass program for nch 128-row bert chunks."""
    import concourse.bacc as bacc
    import concourse.tile as tile
    import concourse.mybir as mybir
    from concourse.bass import MemorySpace
    from concourse.masks import make_identity

    f32 = mybir.dt.float32
    bf16 = mybir.dt.bfloat16

    nc = bacc.Bacc("TRN2", target_bir_lowering=False, debug=False,
                   num_devices=NCORES)

    bert_d = nc.dram_tensor("bertw", [nch * 128, H], bf16,
                            kind="ExternalInput").ap()
    mask_d = nc.dram_tensor("maskp", [128, nch, NMC], bf16,
                            kind="ExternalInput").ap()
    sfac_d = nc.dram_tensor("sfac", [NMC, 1], f32, kind="ExternalInput").ap()
    pron_d = nc.dram_tensor("pronT", [128, HC * BPC], bf16,
                            kind="ExternalInput").ap()
    w1_d = nc.dram_tensor("w1x", [128, KC, HID], bf16,
                          kind="ExternalInput").ap()
    bnb_d = nc.dram_tensor("bnbP", [128, HID // 128], f32,
                           kind="ExternalInput").ap()
    w2_d = nc.dram_tensor("w2P", [128, HID // 128, 3], f32,
                          kind="ExternalInput").ap()
    b2_d = nc.dram_tensor("b2c", [3, 1], f32, kind="ExternalInput").ap()
    out_d = nc.dram_tensor("out", [3, BPC], f32, kind="ExternalOutput").ap()

    with tile.TileContext(nc) as tc:
        with (
            tc.tile_pool(name="consts", bufs=1) as consts,
            tc.tile_pool(name="bert_pool", bufs=1) as bert_pool,
            tc.tile_pool(name="w1_pool", bufs=1) as w1_pool,
            tc.tile_pool(name="head", bufs=1) as head,
            tc.tile_pool(name="ps_mm1", bufs=1, space=MemorySpace.PSUM) as ps_mm1,
            tc.tile_pool(name="ps_tr", bufs=2, space=MemorySpace.PSUM) as ps_tr,
            tc.tile_pool(name="ps_mm2", bufs=1, space=MemorySpace.PSUM) as ps_mm2,
            tc.tile_pool(name="ps_mm3", bufs=1, space=MemorySpace.PSUM) as ps_mm3,
        ):
            # --- constants / small tensors on the ACT HWDGE ring ---
            mask_t = consts.tile([128, nch, NMC], bf16)
            nc.scalar.dma_start(out=mask_t, in_=mask_d)
            sfac_t = consts.tile([NMC, 1], f32)
            nc.scalar.dma_start(out=sfac_t, in_=sfac_d)
            # xT holds x transposed: [128, kc, slot]; kc = e*8 + hc.
            # Pron (e=2 -> kc 16..23) is host-gathered/transposed; DMA it
            # straight into the xT tile.
            xT_t = consts.tile([128, KC, BPC], bf16)
            nc.scalar.dma_start(out=xT_t[:, 2 * HC:3 * HC, :], in_=pron_d)
            bnb_t = head.tile([128, HID // 128], f32)
            nc.scalar.dma_start(out=bnb_t, in_=bnb_d)
            w2_t = head.tile([128, HID // 128, 3], f32)
            nc.scalar.dma_start(out=w2_t, in_=w2_d)
            b2_t = head.tile([3, 1], f32)
            nc.scalar.dma_start(out=b2_t, in_=b2_d)
            idt = consts.tile([NMC, NMC], f32)
            make_identity(nc, idt)

            # --- bulk streams on the SP HWDGE ring: bert FIRST, W1 LAST ---
            bert_t = bert_pool.tile([128, nch, H], bf16)
            bert_src = bert_d.rearrange("(c p) h -> p c h", p=128)
            for c0 in range(0, nch, BERT_PIECE):
                c1 = min(c0 + BERT_PIECE, nch)
                nc.sync.dma_start(out=bert_t[:, c0:c1, :],
                                  in_=bert_src[:, c0:c1, :])
            w1_t = w1_pool.tile([128, KC, HID], bf16)
            for k0 in range(0, KC, W1_PIECE):
                k1 = min(k0 + W1_PIECE, KC)
                nc.sync.dma_start(out=w1_t[:, k0:k1, :],
                                  in_=w1_d[:, k0:k1, :])

            # --- mm1: span sums, masks stationary, bert streaming ---
            ps_lo = ps_mm1.tile([NMC, 512], f32, tag="lo")
            ps_hi = ps_mm1.tile([NMC, 512], f32, tag="hi")
            for ch in range(nch):
                nc.tensor.matmul(ps_lo, mask_t[:, ch, :],
                                 bert_t[:, ch, 0:512],
                                 start=(ch == 0), stop=(ch == nch - 1))
                nc.tensor.matmul(ps_hi, mask_t[:, ch, :],
                                 bert_t[:, ch, 512:H],
                                 start=(ch == 0), stop=(ch == nch - 1))

            # --- scale by 1/span_len (fp32), transpose to xT, cast bf16 ---
            x_sb = consts.tile([NMC, H], f32)
            nc.vector.tensor_scalar_mul(x_sb[:, 0:512], ps_lo, sfac_t)
            nc.vector.tensor_scalar_mul(x_sb[:, 512:H], ps_hi, sfac_t)
            for hc in range(HC):
                pht = ps_tr.tile([128, NMC], f32, tag="pht")
                nc.tensor.transpose(
                    pht, x_sb[:, hc * 128:(hc + 1) * 128], idt)
                nc.vector.tensor_copy(xT_t[:, hc, :], pht[:, 0:BPC])
                nc.vector.tensor_copy(xT_t[:, HC + hc, :], pht[:, BPC:NMC])

            # --- mm2: h[BPC, HID] = x @ W1x over 24 k-chunks ---
            ph = ps_mm2.tile([BPC, HID], f32)
            for kc in range(KC):
                nc.tensor.matmul(ph, xT_t[:, kc, :], w1_t[:, kc, :],
                                 start=(kc == 0), stop=(kc == KC - 1))
            hs_t = head.tile([BPC, HID], f32)
            nc.vector.tensor_copy(hs_t, ph)

            # --- head: transpose h, +BN bias, LeakyReLU, mm3 ---
            ot_ps = ps_mm3.tile([3, BPC], f32, tag="oT")
            for mc in range(HID // 128):
                pht2f = ps_tr.tile([128, NMC], f32, tag="pht")
                pht2 = pht2f[:, 0:BPC]
                nc.tensor.transpose(
                    pht2, hs_t[:, mc * 128:(mc + 1) * 128],
                    idt[0:BPC, 0:BPC])
                t_t = head.tile([128, BPC], f32, tag="t_t")
                nc.vector.tensor_scalar_add(t_t, pht2, bnb_t[:, mc:mc + 1])
                y_t = head.tile([128, BPC], f32, tag="y_t")
                # y = max(0.01 * t, t)
                nc.vector.scalar_tensor_tensor(
                    y_t, t_t, 0.01, t_t,
                    op0=mybir.AluOpType.mult, op1=mybir.AluOpType.max)
                nc.tensor.matmul(
                    ot_ps, w2_t[:, mc, :], y_t,
                    start=(mc == 0), stop=(mc == HID // 128 - 1))

            o_t = head.tile([3, BPC], f32)
            nc.vector.tensor_scalar_add(o_t, ot_ps, b2_t)
            nc.sync.dma_start(out=out_d, in_=o_t)

    nc.compile()
    return nc


def _assign_batches(offs):
    """Union-row counts + balanced assignment of 8 batches to each core.

    Returns (assign[core][slot] -> global batch, nch).
    Constrained LPT: batches sorted by union size desc, each goes to the
    least-loaded core that still has a free slot.
    """
    urows = np.empty(B, dtype=np.int64)
    for b in range(B):
        a0, a1, b0, b1_, _ = (int(v) for v in offs[b])
        la = a1 - a0 + 1
        lb = b1_ - b0 + 1
        ov = max(0, min(a1, b1_) - max(a0, b0) + 1)
        urows[b] = la + lb - ov
    order = np.argsort(-urows, kind="stable")
    loads = np.zeros(NCORES, dtype=np.int64)
    counts = np.zeros(NCORES, dtype=np.int64)
    assign = [[] for _ in range(NCORES)]
    for b in order:
        free = np.where(counts < BPC)[0]
        c = free[np.argmin(loads[free])]
        assign[c].append(int(b))
        loads[c] += urows[b]
        counts[c] += 1
    nch = int(np.ceil(loads.max() / 128))
    return assign, nch


def _prep_core_inputs(bert_f32, offs, batches, nch, w1x, bnbP, w2P, b2c):
    """Build the per-core input map for the given 8 global batch ids."""
    import ml_dtypes
    bf16 = ml_dtypes.bfloat16

    rows_l, slots_l = [], []
    sfac = np.zeros((NMC, 1), dtype=np.float32)
    for s, gb in enumerate(batches):
        a0, a1, b0, b1_, _ = (int(v) for v in offs[gb])
        rows = np.union1d(np.arange(a0, a1 + 1), np.arange(b0, b1_ + 1))
        rows_l.append(rows)
        slots_l.append(np.full(len(rows), s, dtype=np.int64))
        sfac[s, 0] = 1.0 / (a1 - a0 + 1)
        sfac[BPC + s, 0] = 1.0 / (b1_ - b0 + 1)
    rows_cat = np.concatenate(rows_l)
    slots_cat = np.concatenate(slots_l)
    gb_cat = np.asarray(batches)[slots_cat]
    R = len(rows_cat)

    bertw = np.zeros((nch * 128, H), dtype=bf16)
    bertw[:R] = bert_f32[gb_cat, rows_cat].astype(bf16)

    a0s = offs[np.asarray(batches), 0][slots_cat]
    a1s = offs[np.asarray(batches), 1][slots_cat]
    b0s = offs[np.asarray(batches), 2][slots_cat]
    b1s = offs[np.asarray(batches), 3][slots_cat]
    mA = (rows_cat >= a0s) & (rows_cat <= a1s)
    mB = (rows_cat >= b0s) & (rows_cat <= b1s)
    maskflat = np.zeros((nch * 128, NMC), dtype=np.float32)
    idx = np.arange(R)
    maskflat[idx, slots_cat] = mA
    maskflat[idx, BPC + slots_cat] = mB
    maskp = np.ascontiguousarray(
        maskflat.reshape(nch, 128, NMC).transpose(1, 0, 2)).astype(bf16)

    prons = offs[np.asarray(batches), 4]
    pron_rows = bert_f32[np.asarray(batches), prons]  # [BPC, H]
    pronT = np.ascontiguousarray(
        pron_rows.reshape(BPC, HC, 128).transpose(2, 1, 0)
        .reshape(128, HC * BPC)).astype(bf16)

    return {
        "bertw": bertw,
        "maskp": maskp,
        "sfac": sfac,
        "pronT": pronT,
        "w1x": w1x,
        "bnbP": bnbP,
        "w2P": w2P,
        "b2c": b2c,
    }


def kernel(bert_outputs, offsets, W1, b1, gamma, beta, running_mean,
           running_var, W2, b2):
    import ml_dtypes

    bert_f32 = np.ascontiguousarray(np.asarray(bert_outputs, dtype=np.float32))
    offs = np.asarray(offsets).astype(np.int64)
    W1 = np.asarray(W1, dtype=np.float32)
    b1 = np.asarray(b1, dtype=np.float32)
    gamma = np.asarray(gamma, dtype=np.float32)
    beta = np.asarray(beta, dtype=np.float32)
    rm = np.asarray(running_mean, dtype=np.float32)
    rv = np.asarray(running_var, dtype=np.float32)
    W2 = np.asarray(W2, dtype=np.float32)
    b2 = np.asarray(b2, dtype=np.float32)

    # Fold BN eval-mode stats: bn(xW1 + b1) = x(W1*s) + ((b1 - mean)*s + beta)
    s = gamma / np.sqrt(rv + EPS)
    bias = (b1 - rm) * s + beta
    w1f = (W1 * s[None, :]).astype(np.float32)
    # [3H, HID] -> [128 p, kc = e*8 + hc, HID], row = e*1024 + hc*128 + p
    w1x = np.ascontiguousarray(
        w1f.reshape(3, HC, 128, HID).transpose(2, 0, 1, 3)
        .reshape(128, KC, HID)).astype(ml_dtypes.bfloat16)
    bnbP = np.ascontiguousarray(bias.reshape(HID // 128, 128).T)
    w2P = np.ascontiguousarray(
        W2.reshape(HID // 128, 128, 3).transpose(1, 0, 2))
    b2c = np.ascontiguousarray(b2.reshape(3, 1))

    assign, nch = _assign_batches(offs)

    if nch not in _PROGRAM_CACHE:
        _PROGRAM_CACHE[nch] = _build_program(nch)
    nc = _PROGRAM_CACHE[nch]

    in_maps = [
        _prep_core_inputs(bert_f32, offs, assign[c], nch, w1x, bnbP, w2P, b2c)
        for c in range(NCORES)
    ]

    from concourse import bass_utils
    kwargs = {}
    if TRACE:
        kwargs = {"trace": True, "trace_cores": list(range(NCORES))}
    res = bass_utils.run_bass_kernel_spmd(nc, in_maps,
                                          core_ids=list(range(NCORES)),
                                          **kwargs)
    global LAST_RESULT
    LAST_RESULT = res

    out = np.empty((B, 3), dtype=np.float32)
    for c in range(NCORES):
        out[assign[c]] = res.results[c]["out"].T
    return out


# revision 6
# speedup vs baseline: 1.3702x; 1.0272x over previous
"""CorefHead Trainium2 kernel.

Reference computation (B=64, S=512, H=1024, HID=512):
  emb_a = span_mean(bert, offsets[:,0:2])   # [B,H]
  emb_b = span_mean(bert, offsets[:,2:4])   # [B,H]
  emb_p = bert[b, offsets[:,4]]             # [B,H]
  x = concat([emb_a, emb_b, emb_p], -1)     # [B,3H]
  h = leaky_relu(batchnorm_eval(x @ W1 + b1), 0.01)
  out = h @ W2 + b2                         # [B,3]

Strategy: pure data parallel, batch sharded 8 ways (8 batches/core),
DMA-byte and PE-cycle minimized:
  - Host ships ONLY the union of span rows per batch, packed back-to-back
    across the core's 8 batches (no per-batch chunk alignment); the pron
    rows are host-gathered/transposed into a small bf16 block, so they
    never force extra 128-row chunks.
  - mm1 is flipped vs the obvious orientation: the 0/1 span masks
    [128, 16] are the stationary PE operand (LDWEIGHTS of 16 columns is
    ~free) and bert streams as the moving operand at N=512, accumulating
    ALL chunks into one PSUM pair [16, 1024] (cols = 2 spans x 8 slots).
    This cuts PE time ~4x vs loading each bert chunk as weights.
  - x is scaled by 1/span_len in fp32 on the PSUM->SBUF copy, transposed
    to contraction-major via 8 PE transposes, cast to bf16.
  - All small constants ride in TWO packed DMAs (one bf16, one fp32)
    issued first on the ACT ring, so mm1's masks land early.
  - DMA order on the sync ring: bert pieces FIRST, folded W1 (bf16) LAST,
    so mm1 is never starved and mm2 chases the W1 stream.
  - mm2 kc order: pron kcs (16..23, no transpose dependency) first, then
    the 8 transposes, then e0/e1 kcs -- keeps the PE dense (HAM warm)
    instead of ping-ponging with the DVE copies. W1 pieces are shipped
    in the same order mm2 consumes them.
  - Head: per 128-wide hid chunk: PE transpose, +BN bias (folded), leaky
    ReLU, mm3 accumulate [3, 8]; +b2; DMA out.
Host gathers per-core [3, 8] outputs and undoes the batch assignment.
"""

import numpy as np

B, S, H = 64, 512, 1024
HID = 512
EPS = 1e-5
NCORES = 8
BPC = B // NCORES   # batches per core
KC = 3 * H // 128   # 24 contraction chunks for mm2
HC = H // 128       # 8 h-chunks per embedding
NMC = 2 * BPC       # mm1 psum partitions: 2 spans x 8 slots

BERT_PIECE = 3      # bert chunks per DMA piece
# W1 k-chunk pieces, in mm2 consumption order: pron (16..23) then e0/e1.
W1_PIECES = [(16, 20), (20, 24), (0, 4), (4, 8), (8, 12), (12, 16)]
MM2_ORDER = list(range(16, 24)) + list(range(0, 16))

# Test-harness hooks (harness calls kernel() with TRACE=False default).
TRACE = False
LAST_RESULT = None

_PROGRAM_CACHE: dict = {}


def _build_program(nch: int):
    """Build + compile the SPMD Bass program for nch 128-row bert chunks."""
    import concourse.bacc as bacc
    import concourse.tile as tile
    import concourse.mybir as mybir
    from concourse.bass import MemorySpace
    from concourse.masks import make_identity

    f32 = mybir.dt.float32
    bf16 = mybir.dt.bfloat16

    nc = bacc.Bacc("TRN2", target_bir_lowering=False, debug=False,
                   num_devices=NCORES)

    # cbf16: [mask (nch*16) | pronT (64)] per partition.
    # cf32:  [sfac (1) | bnb (4) | w2 (12) | b2 (1)] per partition.
    cbf_d = nc.dram_tensor("cbf", [128, nch * NMC + HC * BPC], bf16,
                           kind="ExternalInput").ap()
    cf32_d = nc.dram_tensor("cf32", [128, 18], f32,
                            kind="ExternalInput").ap()
    bert_d = nc.dram_tensor("bertw", [nch * 128, H], bf16,
                            kind="ExternalInput").ap()
    w1_d = nc.dram_tensor("w1x", [128, KC, HID], bf16,
                          kind="ExternalInput").ap()
    out_d = nc.dram_tensor("out", [3, BPC], f32, kind="ExternalOutput").ap()

    with tile.TileContext(nc) as tc:
        with (
            tc.tile_pool(name="consts", bufs=1) as consts,
            tc.tile_pool(name="bert_pool", bufs=1) as bert_pool,
            tc.tile_pool(name="w1_pool", bufs=1) as w1_pool,
            tc.tile_pool(name="head", bufs=1) as head,
            tc.tile_pool(name="ps_mm1", bufs=1, space=MemorySpace.PSUM) as ps_mm1,
            tc.tile_pool(name="ps_tr", bufs=2, space=MemorySpace.PSUM) as ps_tr,
            tc.tile_pool(name="ps_mm2", bufs=1, space=MemorySpace.PSUM) as ps_mm2,
            tc.tile_pool(name="ps_mm3", bufs=1, space=MemorySpace.PSUM) as ps_mm3,
        ):
            # --- packed constants, 2 DMAs on the ACT HWDGE ring ---
            cbf_t = consts.tile([128, nch * NMC + HC * BPC], bf16)
            nc.scalar.dma_start(out=cbf_t, in_=cbf_d)
            cf32_t = consts.tile([128, 18], f32)
            nc.scalar.dma_start(out=cf32_t, in_=cf32_d)
            mask_t = cbf_t[:, 0:nch * NMC].rearrange(
                "p (c m) -> p c m", c=nch)
            pron_t = cbf_t[:, nch * NMC:].rearrange(
                "p (k s) -> p k s", k=HC)
            sfac_t = cf32_t[0:NMC, 0:1]
            bnb_t = cf32_t[:, 1:5]
            w2_t = cf32_t[:, 5:17].rearrange("p (m j) -> p m j", m=4)
            b2_t = cf32_t[0:3, 17:18]
            idt = consts.tile([NMC, NMC], f32)
            make_identity(nc, idt)

            # --- bulk streams on the SP HWDGE ring: bert FIRST, W1 LAST ---
            bert_t = bert_pool.tile([128, nch, H], bf16)
            bert_src = bert_d.rearrange("(c p) h -> p c h", p=128)
            for c0 in range(0, nch, BERT_PIECE):
                c1 = min(c0 + BERT_PIECE, nch)
                nc.sync.dma_start(out=bert_t[:, c0:c1, :],
                                  in_=bert_src[:, c0:c1, :])
            w1_t = w1_pool.tile([128, KC, HID], bf16)
            for k0, k1 in W1_PIECES:
                nc.sync.dma_start(out=w1_t[:, k0:k1, :],
                                  in_=w1_d[:, k0:k1, :])

            # --- mm1: span sums, masks stationary, bert streaming ---
            ps_lo = ps_mm1.tile([NMC, 512], f32, tag="lo")
            ps_hi = ps_mm1.tile([NMC, 512], f32, tag="hi")
            for ch in range(nch):
                nc.tensor.matmul(ps_lo, mask_t[:, ch, :],
                                 bert_t[:, ch, 0:512],
                                 start=(ch == 0), stop=(ch == nch - 1))
                nc.tensor.matmul(ps_hi, mask_t[:, ch, :],
                                 bert_t[:, ch, 512:H],
                                 start=(ch == 0), stop=(ch == nch - 1))

            # --- mm2 part 1: pron kcs (no transpose dependency) ---
            ph = ps_mm2.tile([BPC, HID], f32)
            for i, kc in enumerate(MM2_ORDER[:HC]):
                nc.tensor.matmul(ph, pron_t[:, kc - 2 * HC, :],
                                 w1_t[:, kc, :], start=(i == 0), stop=False)

            # --- scale by 1/span_len (fp32), transpose to xT, cast bf16 ---
            x_sb = consts.tile([NMC, H], f32)
            nc.vector.tensor_scalar_mul(x_sb[:, 0:512], ps_lo, sfac_t)
            nc.vector.tensor_scalar_mul(x_sb[:, 512:H], ps_hi, sfac_t)
            xT_t = consts.tile([128, 2 * HC, BPC], bf16)
            for hc in range(HC):
                pht = ps_tr.tile([128, NMC], f32, tag="pht")
                nc.tensor.transpose(
                    pht, x_sb[:, hc * 128:(hc + 1) * 128], idt)
                nc.vector.tensor_copy(xT_t[:, hc, :], pht[:, 0:BPC])
                nc.vector.tensor_copy(xT_t[:, HC + hc, :], pht[:, BPC:NMC])

            # --- mm2 part 2: span kcs ---
            for i, kc in enumerate(MM2_ORDER[HC:]):
                nc.tensor.matmul(ph, xT_t[:, kc, :], w1_t[:, kc, :],
                                 start=False, stop=(i == 2 * HC - 1))

            # --- head: transpose h, +BN bias, LeakyReLU, mm3 ---
            hs_t = head.tile([BPC, HID], f32)
            ot_ps = ps_mm3.tile([3, BPC], f32, tag="oT")
            for mc in range(HID // 128):
                nc.vector.tensor_copy(hs_t[:, mc * 128:(mc + 1) * 128],
                                      ph[:, mc * 128:(mc + 1) * 128])
                pht2f = ps_tr.tile([128, NMC], f32, tag="pht")
                pht2 = pht2f[:, 0:BPC]
                nc.tensor.transpose(
                    pht2, hs_t[:, mc * 128:(mc + 1) * 128],
                    idt[0:BPC, 0:BPC])
                t_t = head.tile([128, BPC], f32, tag="t_t")
                nc.vector.tensor_scalar_add(t_t, pht2, bnb_t[:, mc:mc + 1])
                y_t = head.tile([128, BPC], f32, tag="y_t")
                # y = max(0.01 * t, t)
                nc.vector.scalar_tensor_tensor(
                    y_t, t_t, 0.01, t_t,
                    op0=mybir.AluOpType.mult, op1=mybir.AluOpType.max)
                nc.tensor.matmul(
                    ot_ps, w2_t[:, mc, :], y_t,
                    start=(mc == 0), stop=(mc == HID // 128 - 1))

            o_t = head.tile([3, BPC], f32)
            nc.vector.tensor_scalar_add(o_t, ot_ps, b2_t)
            nc.sync.dma_start(out=out_d, in_=o_t)

    nc.compile()
    return nc


def _assign_batches(offs):
    """Union-row counts + balanced assignment of 8 batches to each core.

    Constrained LPT: batches sorted by union size desc, each goes to the
    least-loaded core that still has a free slot.
    """
    urows = np.empty(B, dtype=np.int64)
    for b in range(B):
        a0, a1, b0, b1_, _ = (int(v) for v in offs[b])
        la = a1 - a0 + 1
        lb = b1_ - b0 + 1
        ov = max(0, min(a1, b1_) - max(a0, b0) + 1)
        urows[b] = la + lb - ov
    order = np.argsort(-urows, kind="stable")
    loads = np.zeros(NCORES, dtype=np.int64)
    counts = np.zeros(NCORES, dtype=np.int64)
    assign = [[] for _ in range(NCORES)]
    for b in order:
        free = np.where(counts < BPC)[0]
        c = free[np.argmin(loads[free])]
        assign[c].append(int(b))
        loads[c] += urows[b]
        counts[c] += 1
    nch = int(np.ceil(loads.max() / 128))
    return assign, nch


def _prep_core_inputs(bert_f32, offs, batches, nch, w1x, cf32):
    """Build the per-core input map for the given 8 global batch ids."""
    import ml_dtypes
    bf16 = ml_dtypes.bfloat16

    rows_l, slots_l = [], []
    sfac = np.zeros((128,), dtype=np.float32)
    for s, gb in enumerate(batches):
        a0, a1, b0, b1_, _ = (int(v) for v in offs[gb])
        rows = np.union1d(np.arange(a0, a1 + 1), np.arange(b0, b1_ + 1))
        rows_l.append(rows)
        slots_l.append(np.full(len(rows), s, dtype=np.int64))
        sfac[s] = 1.0 / (a1 - a0 + 1)
        sfac[BPC + s] = 1.0 / (b1_ - b0 + 1)
    rows_cat = np.concatenate(rows_l)
    slots_cat = np.concatenate(slots_l)
    gb_cat = np.asarray(batches)[slots_cat]
    R = len(rows_cat)

    bertw = np.zeros((nch * 128, H), dtype=bf16)
    bertw[:R] = bert_f32[gb_cat, rows_cat].astype(bf16)

    a0s = offs[np.asarray(batches), 0][slots_cat]
    a1s = offs[np.asarray(batches), 1][slots_cat]
    b0s = offs[np.asarray(batches), 2][slots_cat]
    b1s = offs[np.asarray(batches), 3][slots_cat]
    mA = (rows_cat >= a0s) & (rows_cat <= a1s)
    mB = (rows_cat >= b0s) & (rows_cat <= b1s)
    maskflat = np.zeros((nch * 128, NMC), dtype=np.float32)
    idx = np.arange(R)
    maskflat[idx, slots_cat] = mA
    maskflat[idx, BPC + slots_cat] = mB
    maskp = maskflat.reshape(nch, 128, NMC).transpose(1, 0, 2)

    prons = offs[np.asarray(batches), 4]
    pron_rows = bert_f32[np.asarray(batches), prons]  # [BPC, H]
    pronT = pron_rows.reshape(BPC, HC, 128).transpose(2, 1, 0)  # [128,HC,BPC]

    cbf = np.empty((128, nch * NMC + HC * BPC), dtype=bf16)
    cbf[:, 0:nch * NMC] = maskp.reshape(128, nch * NMC).astype(bf16)
    cbf[:, nch * NMC:] = pronT.reshape(128, HC * BPC).astype(bf16)

    cf = cf32.copy()
    cf[:, 0] = sfac

    return {"cbf": cbf, "cf32": cf, "bertw": bertw, "w1x": w1x}


def kernel(bert_outputs, offsets, W1, b1, gamma, beta, running_mean,
           running_var, W2, b2):
    import ml_dtypes

    bert_f32 = np.ascontiguousarray(np.asarray(bert_outputs, dtype=np.float32))
    offs = np.asarray(offsets).astype(np.int64)
    W1 = np.asarray(W1, dtype=np.float32)
    b1 = np.asarray(b1, dtype=np.float32)
    gamma = np.asarray(gamma, dtype=np.float32)
    beta = np.asarray(beta, dtype=np.float32)
    rm = np.asarray(running_mean, dtype=np.float32)
    rv = np.asarray(running_var, dtype=np.float32)
    W2 = np.asarray(W2, dtype=np.float32)
    b2 = np.asarray(b2, dtype=np.float32)

    # Fold BN eval-mode stats: bn(xW1 + b1) = x(W1*s) + ((b1 - mean)*s + beta)
    s = gamma / np.sqrt(rv + EPS)
    bias = (b1 - rm) * s + beta
    w1f = (W1 * s[None, :]).astype(np.float32)
    # [3H, HID] -> [128 p, kc = e*8 + hc, HID], row = e*1024 + hc*128 + p
    w1x = np.ascontiguousarray(
        w1f.reshape(3, HC, 128, HID).transpose(2, 0, 1, 3)
        .reshape(128, KC, HID)).astype(ml_dtypes.bfloat16)

    # packed fp32 consts: [sfac | bnb (4) | w2 (12) | b2 (1)]
    cf32 = np.zeros((128, 18), dtype=np.float32)
    cf32[:, 1:5] = bias.reshape(HID // 128, 128).T
    cf32[:, 5:17] = W2.reshape(HID // 128, 128, 3).transpose(1, 0, 2) \
        .reshape(128, 12)
    cf32[0:3, 17] = b2

    assign, nch = _assign_batches(offs)

    if nch not in _PROGRAM_CACHE:
        _PROGRAM_CACHE[nch] = _build_program(nch)
    nc = _PROGRAM_CACHE[nch]

    in_maps = [
        _prep_core_inputs(bert_f32, offs, assign[c], nch, w1x, cf32)
        for c in range(NCORES)
    ]

    from concourse import bass_utils
    kwargs = {}
    if TRACE:
        kwargs = {"trace": True, "trace_cores": list(range(NCORES))}
    res = bass_utils.run_bass_kernel_spmd(nc, in_maps,
                                          core_ids=list(range(NCORES)),
                                          **kwargs)
    global LAST_RESULT
    LAST_RESULT = res

    out = np.empty((B, 3), dtype=np.float32)
    for c in range(NCORES):
        out[assign[c]] = res.results[c]["out"].T
    return out
